# revision 3
# baseline (speedup 1.0000x reference)
"""Distributed segment-max (BatchPooling) for 8 Trainium2 NeuronCores.

Strategy (data/segment parallel, per the sharding hint):
  - Split the node dim N into 8 contiguous row shards, one per core.
  - On each core, compute the max over every aligned K=64-row block of its
    shard.  Each SBUF partition owns a contiguous run of rows, so the
    per-block max is a free-axis `reduce_max` with a strided access pattern
    (feature dim stride 1, row dim stride D) — no transposes anywhere, and
    the DMAs are large fully-contiguous per-partition reads (64 KiB each).
  - The host folds block maxes into segment maxes.  For the uniform layout
    produced by the reference (segments of 512 = 8 blocks) this is an exact
    reshape+max; for general sorted `batch` the few rows at non-aligned
    segment edges are fixed up from x directly.  max is associative /
    idempotent and involves no arithmetic, so the result is bit-exact.

The per-core kernel reads 64 MiB at ~HBM rate, which is the memory roofline
for this problem (target_regime=memory).
"""

import numpy as np

_P = 128  # SBUF partitions
_D = 128  # feature dim (hardcoded per problem spec)
_K = 64  # rows per device-reduced block
_NCORES = 8
_CHUNK_ROWS = 128  # rows per partition per DMA chunk

_CACHE = {}


def _build_nc(rows_per_core, bufs=2):
    """Raw-bass double-buffered pipeline: SP issues chunk loads, DVE reduces
    each 64-row block along the free axis, SP stores the block maxes once.

    Raw bass (not Tile) because a recycling load DMA needs two waits (WAR on
    DVE + WAW on the previous load) and the PSEUDO_DMA_DIRECT2D lowering
    only supports one inline wait; standalone sequencer `wait_ge`
    instructions sidestep that.  The WAW on a recycled buffer is implied
    transitively: red_sem >= readers-of-that-buffer means those reduces ran,
    and they only ran after observing the previous load's dma_sem increment.
    """
    import concourse.bass as bass
    import concourse.mybir as mybir

    nc = bass.Bass()
    n_blocks = rows_per_core // _K
    rows_per_part = rows_per_core // _P
    n_chunks = rows_per_part // _CHUNK_ROWS
    blocks_per_part = rows_per_part // _K
    bpc = _CHUNK_ROWS // _K  # blocks per chunk per partition

    x = nc.dram_tensor("x", [rows_per_core, _D], mybir.dt.float32, kind="ExternalInput")
    bm = nc.dram_tensor("bm", [n_blocks, _D], mybir.dt.float32, kind="ExternalOutput")

    # Partition p owns rows [p*rows_per_part, (p+1)*rows_per_part).
    xs = x[:].rearrange("(p q) d -> p (q d)", p=_P)
    bo = bm[:].rearrange("(p k) d -> p (k d)", p=_P)

    cw = _CHUNK_ROWS * _D  # elements per partition per chunk

    import contextlib

    with contextlib.ExitStack() as es:
        tiles = es.enter_context(nc.sbuf_tensor([_P, bufs * cw], mybir.dt.float32))
        bmt = es.enter_context(
            nc.sbuf_tensor([_P, blocks_per_part * _D], mybir.dt.float32)
        )
        # One DMA-completion sem per buffer slot: at most one in-flight DMA
        # per sem, so `sem >= 16*(k+1)` exactly means "the k-th load into
        # this slot fully landed" (a single cumulative sem could pass its
        # threshold early if SDMA engines progress unevenly across chunks).
        dma_sems = [
            es.enter_context(nc.semaphore(f"dma_sem{i}")) for i in range(bufs)
        ]
        store_sem = es.enter_context(nc.semaphore("store_sem"))
        red_sem = es.enter_context(nc.semaphore("red_sem"))
        block = es.enter_context(nc.Block())

        @block.sync
        def _(sync):
            for c in range(n_chunks):
                if c >= bufs:
                    # readers of the previous tenant of this slot are done
                    # (which also implies that load fully landed)
                    sync.wait_ge(red_sem, bpc * (c - bufs + 1))
                sync.dma_start(
                    out=tiles[:, (c % bufs) * cw : (c % bufs + 1) * cw],
                    in_=xs[:, c * cw : (c + 1) * cw],
                ).then_inc(dma_sems[c % bufs], 16)
            sync.wait_ge(red_sem, bpc * n_chunks)
            sync.dma_start(out=bo, in_=bmt[:]).then_inc(store_sem, 16)
            sync.wait_ge(store_sem, 16)

        @block.vector
        def _(vector):
            for c in range(n_chunks):
                vector.wait_ge(dma_sems[c % bufs], 16 * (c // bufs + 1))
                view = tiles[
                    :, (c % bufs) * cw : (c % bufs + 1) * cw
                ].rearrange("p (b m d) -> p b d m", b=bpc, m=_K, d=_D)
                for b in range(bpc):
                    k = c * bpc + b
                    nc.vector.reduce_max(
                        out=bmt[:, k * _D : (k + 1) * _D],
                        in_=view[:, b],
                        axis=mybir.AxisListType.X,
                    ).then_inc(red_sem, 1)
    return nc


def _device_block_max(x):
    from concourse.bass_utils import run_bass_kernel_spmd

    n = x.shape[0]
    rows_per_core = n // _NCORES
    if rows_per_core not in _CACHE:
        _CACHE[rows_per_core] = _build_nc(rows_per_core)
    nc = _CACHE[rows_per_core]
    shards = [x[i * rows_per_core : (i + 1) * rows_per_core] for i in range(_NCORES)]
    res = run_bass_kernel_spmd(
        nc, [{"x": s} for s in shards], core_ids=list(range(_NCORES))
    )
    return np.concatenate([r["bm"] for r in res.results], axis=0)


def _combine(bm, x, batch, num_segments):
    n, d = x.shape
    counts = np.bincount(batch, minlength=num_segments)
    starts = np.empty(num_segments + 1, np.int64)
    starts[0] = 0
    np.cumsum(counts, out=starts[1:])

    rows_per_seg = n // num_segments if num_segments else 0
    if (
        num_segments
        and n % num_segments == 0
        and rows_per_seg % _K == 0
        and np.all(counts == rows_per_seg)
    ):
        return np.ascontiguousarray(
            bm.reshape(num_segments, rows_per_seg // _K, d).max(axis=1)
        )

    out = np.full((num_segments, d), -np.inf, dtype=np.float32)
    for s in range(num_segments):
        a, b = int(starts[s]), int(starts[s + 1])
        if a >= b:
            continue
        ca, cb = -(-a // _K), b // _K
        best = None
        if ca < cb:
            best = bm[ca:cb].max(axis=0)
        lo_end = min(b, ca * _K)
        if a < lo_end:
            e = x[a:lo_end].max(axis=0)
            best = e if best is None else np.maximum(best, e)
        hi_start = max(a, cb * _K)
        if hi_start < b:
            e = x[hi_start:b].max(axis=0)
            best = e if best is None else np.maximum(best, e)
        out[s] = best
    return out


def _numpy_segment_max(x, batch, num_segments):
    """Pure-host fallback for inputs the device path doesn't cover
    (unsorted batch, out-of-range ids, unexpected shapes)."""
    out = np.full((num_segments, x.shape[1]), -np.inf, dtype=np.float32)
    if batch.size == 0 or num_segments == 0:
        return out
    keep = (batch >= 0) & (batch < num_segments)
    xb, bb = x[keep], batch[keep]
    order = np.argsort(bb, kind="stable")
    xb, bb = xb[order], bb[order]
    counts = np.bincount(bb, minlength=num_segments)
    starts = np.concatenate([[0], np.cumsum(counts)[:-1]])
    nonempty = counts > 0
    idx = starts[nonempty]
    if idx.size:
        out[nonempty] = np.maximum.reduceat(xb, idx, axis=0)
    return out


def kernel(x, batch, num_segments):
    x = np.ascontiguousarray(np.asarray(x), dtype=np.float32)
    batch = np.asarray(batch)
    num_segments = int(np.asarray(num_segments))
    n, d = x.shape

    in_range = batch.size == 0 or (
        int(batch[0]) >= 0 and int(batch[-1]) < num_segments
    )
    sorted_ok = batch.size == 0 or bool(np.all(batch[1:] >= batch[:-1]))
    shape_ok = d == _D and n == batch.shape[0] and n % (_NCORES * _P * _CHUNK_ROWS) == 0

    if not (shape_ok and sorted_ok and in_range):
        return _numpy_segment_max(x, batch, num_segments)

    bm = _device_block_max(x)
    return _combine(bm, x, batch, num_segments)
